# revision 2
# baseline (speedup 1.0000x reference)
"""DeformConv2d Trainium2 kernel: batch-parallel over 8 NeuronCores.

Per-core redesign vs baseline:
- 18 dma_gathers (tap x position-half) on 4 SWDGE queues; idx streams are
  host-laid-out into per-queue 32-partition windows so no replication DMAs.
- Bilinear-weight flatten (natural [63,448] -> flat [4,9,3200]) done with
  4 stream-order-equivalent DMAs instead of 144 small ones.
- Index pipeline uses trunc-as-floor (valid because clamp floor is 0).
"""
import sys
sys.path.insert(0, '/opt/trn_rl_repo')
import numpy as np

import concourse.bass as bass
from concourse import bacc, mybir
from concourse.alu_op_type import AluOpType
from concourse.tile import TileContext
from concourse.bass_utils import run_bass_kernel_spmd

F16 = mybir.dt.float16
F32 = mybir.dt.float32
I16 = mybir.dt.int16

H = W = 56
C = 64
OC = 64
KK = 9
P = H * W          # 3136
TPAD = 3200        # padded positions per tap
GW = 58            # haloed grid width (y,x in [-1,56])
TVIEW = 3392
TROWS = 3456
NCOLR = P // 16    # 196 real idx cols per tap
NCOLP = TPAD // 16  # 200 idx cols per tap
HALF_A = 1792      # positions 0:1792   (quarters 0,1)
HALF_B = 1408      # positions 1792:3200 (quarters 2,3 + pad)
COLS_A = HALF_A // 16  # 112
QN = 896           # quarter width (position cols per out_ps tile)
CHUNK = 448


# ---------------- host prep ----------------

def build_t2(x_img: np.ndarray) -> np.ndarray:
    """x_img [C,H,W] f32 -> T2 [TROWS,128] f16 pair table with zero halo."""
    xh = x_img.astype(np.float16)
    t2 = np.zeros((TROWS, 128), dtype=np.float16)
    grid = t2[:GW * GW].reshape(GW, GW, 128)
    grid[1:57, 1:57, 0:C] = xh.transpose(1, 2, 0)       # value at (y,x), y=gy-1
    grid[0:56, 1:57, C:2 * C] = xh.transpose(1, 2, 0)   # value at (y+1,x)
    return t2


def _grids():
    """Grid coords WITHOUT +8 shift: gy = oy+ky (= y+1), gx = ox+kx."""
    ky, kx = np.meshgrid(np.arange(3), np.arange(3), indexing='ij')
    ky = ky.reshape(KK)
    kx = kx.reshape(KK)
    oy, ox = np.meshgrid(np.arange(H), np.arange(W), indexing='ij')
    gy = (oy[None] + ky[:, None, None]).astype(np.float32)  # [K,H,W]
    gx = (ox[None] + kx[:, None, None]).astype(np.float32)
    return gy.reshape(KK, P), gx.reshape(KK, P)


def _natmask():
    m = np.ones((128, 448), dtype=np.float32)
    m[63] = 0.0
    m[127] = 0.0
    return m


def _wrap1(pl):  # [P] f32 -> [16, 196]
    return pl.reshape(NCOLR, 16).T.copy()


def host_inputs(x_img, off_img, weight):
    gy, gx = _grids()
    offp = off_img.reshape(KK, 2, P)
    py = offp[:, 0] + gy   # [K, P] grid-y float coords
    px = offp[:, 1] + gx

    # natural layout for the weight pipeline: rows k*7+r (y), 64+k*7+r (x)
    nat = np.zeros((128, 448), dtype=np.float32)
    nat[0:63] = py.reshape(KK * 7, 448)
    nat[64:127] = px.reshape(KK * 7, 448)
    nat *= _natmask()

    # windowed wrapped idx streams [128, 1200]: tap k -> queue q=k%4 slot s=k//4
    # partitions [32q:32q+16] and [32q+16:32q+32] identical;
    # y cols [200s:200s+196], x cols [600+200s : 600+200s+196]; pad cols = 0.
    idxw = np.zeros((128, 1200), dtype=np.float32)
    for k in range(KK):
        q, s = k % 4, k // 4
        wy = _wrap1(py[k])
        wx = _wrap1(px[k])
        for hwin in range(2):
            p0 = 32 * q + 16 * hwin
            idxw[p0:p0 + 16, 200 * s:200 * s + 196] = wy
            idxw[p0:p0 + 16, 600 + 200 * s:600 + 200 * s + 196] = wx

    wt = weight.reshape(OC, C, KK).transpose(1, 2, 0)  # [C, K, OC]
    wlhs = np.concatenate([wt, wt], axis=0).astype(np.float16)  # [128, K, OC]
    rlhs = np.zeros((2, 128), dtype=np.float16)
    rlhs[0, 0:64] = 1.0
    rlhs[1, 64:128] = 1.0
    return {
        "t2": build_t2(x_img),
        "nat": nat,
        "idxw": idxw,
        "wlhs": np.ascontiguousarray(wlhs),
        "rlhs": rlhs,
    }


# ---------------- device kernel ----------------

def gen_kernel(n_cores=8, loop_n=None):
    nc = bacc.Bacc("TRN2", target_bir_lowering=False, debug=False,
                   num_devices=n_cores, num_swdge_queues=4)

    t2 = nc.dram_tensor("t2", [TROWS, 128], F16, kind="ExternalInput")
    nat_in = nc.dram_tensor("nat", [128, 448], F32, kind="ExternalInput")
    idxw_in = nc.dram_tensor("idxw", [128, 1200], F32, kind="ExternalInput")
    wlhs = nc.dram_tensor("wlhs", [128, KK, OC], F16, kind="ExternalInput")
    rlhs = nc.dram_tensor("rlhs", [2, 128], F16, kind="ExternalInput")
    out = nc.dram_tensor("out", [OC, P], F32, kind="ExternalOutput")

    with TileContext(nc) as tc:
        with tc.tile_pool(name="const", bufs=1) as const, \
             tc.tile_pool(name="pipe", bufs=8) as pipe, \
             tc.tile_pool(name="ppro", bufs=1) as ppro, \
             tc.tile_pool(name="gpool", bufs=11) as gpool, \
             tc.tile_pool(name="upool", bufs=4) as upool, \
             tc.tile_pool(name="wtpool", bufs=4) as wtpool, \
             tc.tile_pool(name="opool", bufs=2) as opool, \
             tc.tile_pool(name="psw", bufs=2, space="PSUM") as psw, \
             tc.tile_pool(name="pso", bufs=2, space="PSUM") as pso:

            def ctile(shape, dt, tag):
                return const.tile(shape, dt, tag=tag, name=tag)

            # ---- load constants / inputs ----
            wlhs_sb = ctile([128, KK, OC], F16, "wlhs_sb")
            nc.sync.dma_start(out=wlhs_sb[:], in_=wlhs.ap())
            rlhs_sb = ctile([2, 128], F16, "rlhs_sb")
            nc.sync.dma_start(out=rlhs_sb[:], in_=rlhs.ap())
            nat_sb = ctile([128, 448], F32, "nat_sb")
            nc.sync.dma_start(out=nat_sb[:], in_=nat_in.ap())
            idxw_sb = ctile([128, 1200], F32, "idxw_sb")
            nc.sync.dma_start(out=idxw_sb[:], in_=idxw_in.ap())

            loop_ctx = tc.For_i(0, loop_n, 1) if loop_n else None
            import contextlib
            with (loop_ctx if loop_ctx is not None else contextlib.nullcontext()):
                def pt(tag, shape=(128, 448), dt=F32):
                    return pipe.tile(list(shape), dt, tag=tag, name=tag)

                # ---- idx pipeline: trunc == floor after clamp to [0, 57] ----
                fiw = pt("fiw", (128, 1200), I16)
                nc.vector.tensor_copy(out=fiw[:], in_=idxw_sb[:])
                cyw = pt("cyw", (128, 600), I16)
                nc.vector.tensor_scalar(out=cyw[:], in0=fiw[:, 0:600],
                                        scalar1=0, scalar2=57,
                                        op0=AluOpType.max, op1=AluOpType.min)
                cxw = pt("cxw", (128, 600), I16)
                nc.vector.tensor_scalar(out=cxw[:], in0=fiw[:, 600:1200],
                                        scalar1=0, scalar2=57,
                                        op0=AluOpType.max, op1=AluOpType.min)
                ji = pt("ji", (128, 600), I16)
                nc.vector.scalar_tensor_tensor(out=ji[:], in0=cyw[:], scalar=float(GW),
                                               in1=cxw[:], op0=AluOpType.mult,
                                               op1=AluOpType.add)

                # ---- gathers: tap k, halves (A: 0:1792, B: 1792:3200) ----
                t2full = t2.ap()
                t2view = bass.AP(tensor=t2full.tensor, offset=t2full.offset,
                                 ap=[[128, TVIEW], [1, 256]])
                gt = {}
                for k in range(KK):
                    q, s = k % 4, k // 4
                    c0 = 200 * s
                    gA = gpool.tile([128, 2, HALF_A], F16, tag="g", name=f"gA{k}")
                    nc.gpsimd.dma_gather(
                        gA[:], t2view, ji[:, c0:c0 + COLS_A],
                        HALF_A, HALF_A, 256, elem_step=128, transpose=True,
                        queue_num=q)
                    gB = gpool.tile([128, 2, HALF_B], F16, tag="g", name=f"gB{k}")
                    nc.gpsimd.dma_gather(
                        gB[:], t2view, ji[:, c0 + COLS_A:c0 + NCOLP],
                        HALF_B, HALF_B, 256, elem_step=128, transpose=True,
                        queue_num=q)
                    gt[k] = (gA, gB)

                # ---- weight pipeline (natural layout) ----
                # pf = nat (grid+off pre-added on host); true floor needed here.
                fi = pt("fi", (128, 448), I16)
                nc.vector.tensor_copy(out=fi[:], in_=nat_sb[:])
                fr = pt("fr")
                nc.vector.tensor_copy(out=fr[:], in_=fi[:])
                dd = pt("dd")
                nc.vector.tensor_tensor(out=dd[:], in0=nat_sb[:], in1=fr[:],
                                        op=AluOpType.subtract)
                ng = pt("ng")
                nc.vector.tensor_scalar(out=ng[:], in0=dd[:], scalar1=0.0,
                                        scalar2=None, op0=AluOpType.is_lt)
                ff = pt("ff")
                nc.vector.tensor_tensor(out=ff[:], in0=fr[:], in1=ng[:],
                                        op=AluOpType.subtract)
                tt = pt("tt")
                nc.vector.tensor_tensor(out=tt[:], in0=nat_sb[:], in1=ff[:],
                                        op=AluOpType.subtract)
                # masks: top neighbor valid iff ff-1 in [0,55] -> ff in [1,56]
                #        bottom neighbor valid iff ff in [0,55]
                a0 = pt("a0")
                nc.vector.tensor_scalar(out=a0[:], in0=ff[:], scalar1=1.0,
                                        scalar2=None, op0=AluOpType.is_ge)
                b0 = pt("b0")
                nc.vector.tensor_scalar(out=b0[:], in0=ff[:], scalar1=56.0,
                                        scalar2=None, op0=AluOpType.is_le)
                m0 = pt("m0")
                nc.vector.tensor_tensor(out=m0[:], in0=a0[:], in1=b0[:],
                                        op=AluOpType.mult)
                a1 = pt("a1")
                nc.vector.tensor_scalar(out=a1[:], in0=ff[:], scalar1=0.0,
                                        scalar2=None, op0=AluOpType.is_ge)
                b1 = pt("b1")
                nc.vector.tensor_scalar(out=b1[:], in0=ff[:], scalar1=55.0,
                                        scalar2=None, op0=AluOpType.is_le)
                m1 = pt("m1")
                nc.vector.tensor_tensor(out=m1[:], in0=a1[:], in1=b1[:],
                                        op=AluOpType.mult)
                onemt = pt("onemt")
                nc.vector.tensor_scalar(out=onemt[:], in0=tt[:], scalar1=1.0,
                                        scalar2=-1.0, op0=AluOpType.subtract,
                                        op1=AluOpType.mult)
                w0 = pt("w0")
                nc.vector.tensor_tensor(out=w0[:], in0=onemt[:], in1=m0[:],
                                        op=AluOpType.mult)
                w1 = pt("w1")
                nc.vector.tensor_tensor(out=w1[:], in0=tt[:], in1=m1[:],
                                        op=AluOpType.mult)
                w0x = pt("w0x", (63, 448))
                nc.vector.tensor_copy(out=w0x[:], in_=w0[64:127, :])
                w1x = pt("w1x", (63, 448))
                nc.vector.tensor_copy(out=w1x[:], in_=w1[64:127, :])

                # products, f16 out directly; order = rep partition order
                # rep partitions: 0=w00 (top,left) 1=w10 (bot,left)
                #                 2=w01 (top,right) 3=w11 (bot,right)
                wprod = []
                for nm, (wy, wx) in (("w00", (w0, w0x)), ("w10", (w1, w0x)),
                                     ("w01", (w0, w1x)), ("w11", (w1, w1x))):
                    th = pipe.tile([63, 448], F16, tag=nm, name=nm)
                    nc.vector.tensor_tensor(out=th[:], in0=wy[0:63, :], in1=wx[:],
                                            op=AluOpType.mult)
                    wprod.append(th)

                # ---- flatten: 4 stream-order-equivalent DMAs ----
                rep = ppro.tile([4, KK, TPAD], F16, tag="rep", name="rep")
                nc.vector.memset(rep[:, :, P:TPAD], 0.0)
                for i, th in enumerate(wprod):
                    nc.sync.dma_start(out=rep[i:i + 1, :, 0:P], in_=th[:])

                # ---- main loop: position-quarters x taps x 448-chunks ----
                for Q in range(4):
                    q0 = QN * Q                      # global col base
                    qn = QN if Q < 3 else P - q0     # 896 or 448
                    half = 0 if Q < 2 else 1
                    hbase = 0 if half == 0 else HALF_A
                    out_ps = pso.tile([OC, QN], F32, tag="out_ps", name="out_ps")
                    nchunk = (qn + CHUNK - 1) // CHUNK
                    for k in range(KK):
                        g = gt[k][half]
                        for c in range(nchunk):
                            cg = q0 + CHUNK * c          # global col
                            cl = cg - hbase              # col within half tile
                            wt_ps = psw.tile([128, 2, CHUNK], F32, tag="wtps",
                                             name="wtps",
                                             padded_shape=[128, 2, 512])
                            for s in range(2):
                                nc.tensor.matmul(wt_ps[:, s, :], rlhs_sb[:],
                                                 rep[2 * s:2 * s + 2, k, cg:cg + CHUNK],
                                                 start=True, stop=True)
                            wt_sb = wtpool.tile([128, 2, CHUNK], F16, tag="wtsb",
                                                name="wtsb")
                            nc.scalar.copy(out=wt_sb[:], in_=wt_ps[:])
                            u = upool.tile([128, 2, CHUNK], F16, tag="u", name="u")
                            nc.vector.tensor_tensor(out=u[:],
                                                    in0=g[:, :, cl:cl + CHUNK],
                                                    in1=wt_sb[:],
                                                    op=AluOpType.mult)
                            for s in range(2):
                                nc.tensor.matmul(
                                    out_ps[:, CHUNK * c:CHUNK * c + CHUNK],
                                    wlhs_sb[:, k, :], u[:, s, :],
                                    start=(k == 0 and s == 0),
                                    stop=(k == KK - 1 and s == 1))
                    osb = opool.tile([OC, QN], F32, tag="osb", name="osb")
                    nc.vector.tensor_copy(out=osb[:, 0:qn], in_=out_ps[:, 0:qn])
                    nc.sync.dma_start(out=out.ap()[:, q0:q0 + qn],
                                      in_=osb[:, 0:qn])

    nc.compile()
    return nc


# ---------------- runners ----------------

LAST_EXEC_NS = None


def kernel(input, offset, weight):
    """Full-batch DeformConv2d on 8 NeuronCores (batch-parallel)."""
    x = np.asarray(input, dtype=np.float32)
    off = np.asarray(offset, dtype=np.float32)
    wt = np.asarray(weight, dtype=np.float32)
    B = x.shape[0]
    nc = gen_kernel(B)
    in_maps = [host_inputs(x[b], off[b], wt) for b in range(B)]
    res = run_bass_kernel_spmd(nc, in_maps, core_ids=list(range(B)))
    global LAST_EXEC_NS
    LAST_EXEC_NS = res.exec_time_ns
    out = np.stack([np.asarray(r["out"]).reshape(OC, H, W) for r in res.results])
    return out.astype(np.float32)
